# revision 5
# baseline (speedup 1.0000x reference)
"""Trainium2 Bass kernel for nn_Pre_loss_53566832116190 (topk_masking).

Strategy: data-parallel over batch N across 8 NeuronCores. Each core
computes the per-row KL criterion for its shard of rows for both the
(pred_x, gt_x) and (pred_y, gt_y) pairs:

    loss[r] = ( S/s_l - ln(s_l) + ln(s_p) ) / W
      with e_l = exp(l), s_l = sum(e_l), s_p = sum(exp(p)),
           S = sum(e_l * (l - p))

which is algebraically KLDiv(log_softmax(p), softmax(l)).mean(axis=-1).
The small [N*K] merge (global top-k mask, weights, weighted loss sum)
is done on the host after gathering per-core loss rows.
"""

import numpy as np

_N, _K, _W = 1024, 17, 512
_NCORES = 8
_NB = _N // _NCORES      # 128 batch rows per core
_M = _NB * _K            # 2176 loss rows per core
_T = _M // 128           # 17 row-tiles of 128 rows
_COLS = 2 * _T           # x tiles then y tiles

_nc_cache = None
last_results = None  # BassKernelResults of the most recent run (for test harness)


def _build_module():
    import concourse.bacc as bacc
    import concourse.mybir as mybir
    import concourse.tile as tile

    f32 = mybir.dt.float32
    Exp = mybir.ActivationFunctionType.Exp
    Ln = mybir.ActivationFunctionType.Ln
    mult = mybir.AluOpType.mult
    add = mybir.AluOpType.add

    # Bacc (not plain Bass): its finalize() runs generate_event_semaphores,
    # which splits multi-wait sync conditions that TRN2 structs can't hold.
    nc = bacc.Bacc("TRN2", debug=False)

    dram = {}
    for name in ("pred_x", "gt_x", "pred_y", "gt_y"):
        dram[name] = nc.dram_tensor(name, [_M, _W], f32, kind="ExternalInput").ap()
    out_d = nc.dram_tensor("loss_out", [128, _COLS], f32, kind="ExternalOutput").ap()

    # row-tile groups of up to 4 (1 MiB DMA per tensor per group)
    groups = []
    t0 = 0
    while t0 < _T:
        gs = min(4, _T - t0)
        groups.append((t0, gs))
        t0 += gs

    with tile.TileContext(nc) as tc:
        with (
            tc.tile_pool(name="io", bufs=3) as io_pool,
            tc.tile_pool(name="work", bufs=3) as work_pool,
            tc.tile_pool(name="acc", bufs=1) as acc_pool,
        ):
            sl_acc = acc_pool.tile([128, _COLS], f32)  # sum exp(l)
            sp_acc = acc_pool.tile([128, _COLS], f32)  # sum exp(p)
            ss_acc = acc_pool.tile([128, _COLS], f32)  # sum e_l*(l-p)

            pairs = (("pred_x", "gt_x"), ("pred_y", "gt_y"))
            for pi, (pname, lname) in enumerate(pairs):
                # row r = t*128 + p  ->  [p, t, w]
                p_r = dram[pname].rearrange("(t p) w -> p t w", p=128)
                l_r = dram[lname].rearrange("(t p) w -> p t w", p=128)
                for (t0, gs) in groups:
                    lt = io_pool.tile([128, gs, _W], f32, tag="lt")
                    nc.sync.dma_start(out=lt, in_=l_r[:, t0 : t0 + gs, :])
                    pt = io_pool.tile([128, gs, _W], f32, tag="pt")
                    nc.sync.dma_start(out=pt, in_=p_r[:, t0 : t0 + gs, :])

                    dt_ = work_pool.tile([128, gs, _W], f32, tag="dt")
                    nc.vector.tensor_sub(dt_, lt, pt)
                    el = work_pool.tile([128, gs, _W], f32, tag="el")
                    ep = work_pool.tile([128, gs, _W], f32, tag="ep")
                    col = pi * _T + t0
                    for j in range(gs):
                        nc.scalar.activation(
                            el[:, j, :], lt[:, j, :], Exp,
                            accum_out=sl_acc[:, col + j : col + j + 1],
                        )
                        nc.scalar.activation(
                            ep[:, j, :], pt[:, j, :], Exp,
                            accum_out=sp_acc[:, col + j : col + j + 1],
                        )
                    # ep reused as product scratch (its accums are captured)
                    nc.vector.tensor_mul(ep, el, dt_)
                    nc.vector.tensor_reduce(
                        out=ss_acc[:, col : col + gs],
                        in_=ep,
                        axis=mybir.AxisListType.X,
                        op=add,
                    )

            res_t = acc_pool.tile([128, _COLS], f32)
            nc.vector.reciprocal(res_t, sl_acc)           # 1/s_l
            nc.vector.tensor_mul(res_t, ss_acc, res_t)    # S/s_l
            ln_sl = acc_pool.tile([128, _COLS], f32)
            nc.scalar.activation(ln_sl, sl_acc, Ln)
            ln_sp = acc_pool.tile([128, _COLS], f32)
            nc.scalar.activation(ln_sp, sp_acc, Ln)
            nc.vector.tensor_sub(res_t, res_t, ln_sl)
            nc.vector.tensor_add(res_t, res_t, ln_sp)
            out_t = acc_pool.tile([128, _COLS], f32)
            nc.scalar.mul(out_t, res_t, 1.0 / _W)
            nc.sync.dma_start(out=out_d, in_=out_t)

    nc.finalize()
    return nc


def get_module():
    global _nc_cache
    if _nc_cache is None:
        _nc_cache = _build_module()
    return _nc_cache


def _host_finish(loss_x, loss_y, target_weight, use_labels, epoch):
    """Replicates reference's cheap [N*K]-sized tail exactly (numpy)."""
    tw = np.asarray(target_weight, dtype=np.float32)
    ul = np.asarray(use_labels)
    weight_real = (tw * (ul == 0).astype(np.float32)[:, None]).reshape(-1)

    cur = float(np.clip(int(epoch) - 210, 0.0, 30.0))
    r = 0.5 * (np.cos(np.pi * cur / 30.0) + 1.0)
    rate = float(np.clip(r, 0.8, 1.0))
    num_visible = int(np.count_nonzero(tw))
    k = int(num_visible * rate)

    tw_flat = tw.reshape(-1)
    loss_all = 0.0
    weights = []
    for loss_small in (loss_x, loss_y):
        loss_new = np.where(tw_flat > 0, loss_small, loss_small.max())
        # k smallest values; ties broken toward lower index (matches
        # jax.lax.top_k on the negated vector)
        idx = np.argsort(loss_new, kind="stable")[:k]
        mask = np.zeros(_N * _K, dtype=np.float32)
        mask[idx] = 1.0
        weight_all = (np.float32(2.0) * weight_real + mask).astype(np.float32)
        weights.append(weight_all)
        loss_all += float(
            np.sum(loss_small.astype(np.float64) * weight_all.astype(np.float64))
        )
    loss = np.float32(loss_all / _K)
    return (np.asarray(loss, dtype=np.float32), weights[0], weights[1])


def _ensure_axon_hooks_importable():
    # concourse.bass_utils imports antenv.axon_hooks when BASS_TRACE is set
    # under axon; some containers ship an antenv stub without it.
    try:
        import antenv.axon_hooks  # noqa: F401
    except Exception:
        import sys
        import types

        m = types.ModuleType("antenv.axon_hooks")
        m._hook = None
        m.set_axon_ntff_profile_hook = lambda hook: setattr(m, "_hook", hook)
        m.get_axon_ntff_profile_hook = lambda: m._hook
        sys.modules["antenv.axon_hooks"] = m


def kernel(pred_x, pred_y, gt_x, gt_y, target_weight, use_labels, epoch):
    global last_results
    _ensure_axon_hooks_importable()
    from concourse import bass_utils

    pred_x = np.ascontiguousarray(np.asarray(pred_x, dtype=np.float32))
    pred_y = np.ascontiguousarray(np.asarray(pred_y, dtype=np.float32))
    gt_x = np.ascontiguousarray(np.asarray(gt_x, dtype=np.float32))
    gt_y = np.ascontiguousarray(np.asarray(gt_y, dtype=np.float32))

    nc = get_module()
    in_maps = []
    for c in range(_NCORES):
        s = slice(c * _NB, (c + 1) * _NB)
        in_maps.append(
            {
                "pred_x": np.ascontiguousarray(pred_x[s].reshape(_M, _W)),
                "gt_x": np.ascontiguousarray(gt_x[s].reshape(_M, _W)),
                "pred_y": np.ascontiguousarray(pred_y[s].reshape(_M, _W)),
                "gt_y": np.ascontiguousarray(gt_y[s].reshape(_M, _W)),
            }
        )

    res = bass_utils.run_bass_kernel_spmd(nc, in_maps, core_ids=list(range(_NCORES)))
    last_results = res

    loss_x = np.empty((_NCORES, _T, 128), dtype=np.float32)
    loss_y = np.empty((_NCORES, _T, 128), dtype=np.float32)
    for c, r in enumerate(res.results):
        o = r["loss_out"]  # [128, 2T]; [p, t] = row t*128+p of this shard
        loss_x[c] = o[:, :_T].T
        loss_y[c] = o[:, _T:].T

    return _host_finish(
        loss_x.reshape(-1), loss_y.reshape(-1), target_weight, use_labels, epoch
    )
